# revision 25
# baseline (speedup 1.0000x reference)
"""Trainium2 Bass kernel for nn_DiffusionDecoder (diffusion decoder losses).

Computes (loss_diffusion, loss_species, l_repulsion) from full inputs,
data-parallel over crystals across 8 NeuronCores.

Structure (per core):
  Species head (atoms globally sorted by species on host):
    - mm1: hidden_pre = W1^T @ h  (PE, bf16, w1 stationary)
    - silu on ACT (single pass, silu table set), hidden -> SBUF bf16
    - mm2 per 128-atom tile: logits (100) + windowed pick logits (4)
      into one PSUM row (PE, per-tile stationary hidden)
    - exp on ACT (natural_log_exp set) -> E bf16, zero-padded to 104
    - sumexp via bf16 fold tree + tensor_reduce (DVE), ln+accum (ACT)
    - pick: host-built one-hot window mask, tensor_tensor_reduce (DVE)
  Repulsion (pred_x0 precomputed+wrapped on host, comp0 pre-scaled by
  sqrt(G00); per-crystal quadratic-form scalars via Gram grouping):
    - 3 wrapped pairwise diff passes (custom DVE, k=1..32 circular)
    - quadratic: SQLC_PLUS + LC2 + VSQ customs + one bf16 add
    - dist = exp(0.5*ln(d2+eps)) on ACT (stays in natural_log_exp set)
    - rep tail (0.8-d)^2 for d<0.8 with accum (custom DVE)
  MSE and final scalar assembly on host.
"""
import numpy as np
import ml_dtypes

import concourse.bass as bass
import concourse.bacc as bacc
import concourse.tile as tile
from concourse import mybir
from concourse.bass_utils import run_bass_kernel_spmd
from concourse.bass_types import AP as _AP

import operator
import concourse.dve_ops as dve_ops
from concourse.dve_ops import DveOp
from concourse.dve_spec import (C0, C1, C2, AluOp, Bin, Spec, Src0, Src1, Zero,
                                lower as _dve_lower, relu as _relu,
                                select as _select, sq as _sq,
                                _has_src1 as _dve_has_src1)
from concourse.dve_uop import DveOpSpec


def _register_dve_op(name, spec):
    """Register a new custom DVE op at runtime (sha computed, not pinned)."""
    if name in dve_ops._SUB_OPCODE_FOR_NAME:
        return next(o for o in dve_ops.OPS if o.name == name)
    row = dve_ops._CUSTOM_DVE_ROW_BASE + len(dve_ops.OPS)
    assert row < 0x20
    dve_ops._SUB_OPCODE_FOR_NAME[name] = row
    shas = {}
    for ver in ("v3", "v4"):
        s = DveOpSpec(name=name, opcode=row, uops=_dve_lower(spec, ver=ver),
                      rd1_en=_dve_has_src1(spec))
        shas[ver] = s.sha(ver)
    op = DveOp(name, spec, subdim=False, uops_sha=shas)
    dve_ops.OPS.append(op)
    dve_ops.CUSTOM_DVE_SPECS[name] = spec
    return op


def _sub(a, b):
    return Bin(AluOp.SUBTRACT, a, b)


def _lt(a, b):
    return Bin(AluOp.IS_LT, a, b)


def _gt(a, b):
    return Bin(AluOp.IS_LT, b, a)


def _mul(a, b):
    return Bin(AluOp.MULTIPLY, a, b)


_d = _sub(Src0, Src1)
# w = d + 1*((d < -s0) - (d > s0)); s0 = half period (unscaled comps)
WRAP_DIFF = _register_dve_op(
    "ANT_WRAP_DIFF",
    Spec(body=_d + _sub(_lt(_d, _sub(Zero, C0)), _gt(_d, C0)),
         reference=lambda in0, in1, s0, s1, imm2: (
             (in0.astype(np.float32) - in1)
             + (((in0.astype(np.float32) - in1) < -s0).astype(np.float32)
                - ((in0.astype(np.float32) - in1) > s0).astype(np.float32)))))
# w = s0*(d + ((d < -s1) - (d > s1))); scale s0 per-partition, s1 literal
WRAP_SC = _register_dve_op(
    "ANT_WRAP_SC",
    Spec(body=_mul(_d + _sub(_lt(_d, _sub(Zero, C1)), _gt(_d, C1)), C0),
         reference=lambda in0, in1, s0, s1, imm2: (
             (np.asarray(s0, np.float32).reshape(
                 in0.shape[0], *([1] * (in0.ndim - 1)))
              if np.ndim(s0) and np.size(s0) > 1 else np.float32(s0))
             * ((in0.astype(np.float32) - in1)
                + (((in0.astype(np.float32) - in1) < -s1).astype(np.float32)
                   - ((in0.astype(np.float32) - in1) > s1).astype(np.float32))))))
LC2 = _register_dve_op(
    "ANT_LC2",
    Spec(body=Src0 * C0 + Src1 * C1,
         reference=lambda in0, in1, s0, s1, imm2: (
             in0.astype(np.float32) * s0 + in1 * s1)))
SQLC_PLUS = _register_dve_op(
    "ANT_SQLC_PLUS",
    Spec(body=_sq(Src0 + Src1 * C0) + _sq(Src1) * C1,
         reference=lambda in0, in1, s0, s1, imm2: (
             (in0.astype(np.float32) + in1 * s0) ** 2
             + in1.astype(np.float32) ** 2 * s1)))
# v = in0*in1 + s0*in1^2
VSQ = _register_dve_op(
    "ANT_VSQ",
    Spec(body=_mul(Src0, Src1) + _sq(Src1) * C0,
         reference=lambda in0, in1, s0, s1, imm2: (
             in0.astype(np.float32) * in1 + s0 * in1.astype(np.float32) ** 2)))


def _vsqw_ref(in0, in1, s0, s1, imm2):
    d = in1.astype(np.float32)
    w = d + ((d < -s1).astype(np.float32) - (d > s1).astype(np.float32))
    s0r = np.asarray(s0, np.float32)
    if s0r.ndim and s0r.size > 1:
        s0r = s0r.reshape(in0.shape[0], *([1] * (in0.ndim - 1)))
    return in0.astype(np.float32) * w + s0r * w * w


_w = Src1 + _sub(_lt(Src1, _sub(Zero, C1)), _gt(Src1, C1))
# v = in0*wrap(in1) + s0*wrap(in1)^2; s1 = wrap threshold (literal 0.5)
VSQW = _register_dve_op(
    "ANT_VSQW",
    Spec(body=_mul(Src0, _w) + _sq(_w) * C0,
         reference=_vsqw_ref))


def _rep_tail_ref(in0, in1, s0, s1, imm2):
    a = in0.astype(np.float32)
    b = np.where(a < s1, (s1 - a) ** 2, 0.0).astype(np.float32)
    return b, s0 + b.reshape(b.shape[0], -1).sum(axis=-1, keepdims=True)


REP_TAIL = _register_dve_op(
    "ANT_REP_TAIL",
    Spec(body=_select(_lt(Src0, C1), _sq(_sub(C1, Src0)), Zero),
         accum=operator.add, accum_init=C0,
         reference=_rep_tail_ref))

# Steer walrus's activation-table chooser: Exp/Ln must both resolve to the
# combined natural_log_exp set, not the separate exp/ln sets (saves ~5 table
# loads = ~13us ACT per rep). Names/positions preserved; entries emptied.
import functools as _ft
import concourse.bacc as _bacc_mod
import concourse.hw_specs as _hw_specs

@_ft.cache
def _ant_tables(arch):
    t = {k: set(v) for k, v in _hw_specs.get_activation_tables(arch).items()}
    for k in ("exp_and_others", "natural_log"):
        if k in t:
            t[k] = set()
    return t

_bacc_mod.get_activation_tables = _ant_tables

F32 = mybir.dt.float32
BF16 = mybir.dt.bfloat16
AF = mybir.ActivationFunctionType
OP = mybir.AluOpType

TIMESTEPS = 1000
B = 2048
NPER = 64
N = B * NPER
D = 64            # node dim
H = 128           # hidden dim
C = 100           # species
CP = 104          # padded class dim (bf16 fold alignment)
W = 4             # pick window width
NCORES = 8
B_LOC = B // NCORES            # 256 crystals / core
N_LOC = N // NCORES            # 16384 atoms / core
NT = N_LOC // 128              # 128 atom tiles / core
NG = 8                         # logits groups
TPG = NT // NG                 # 16 tiles per group
CT = B_LOC // 128              # 2 crystal tiles / core
FCH = 2048                     # mm1 chunk (4 PSUM banks)
NCH = N_LOC // FCH


def _shift_pairs_ap(tile_ap, comp):
    """[128, 64, 32] view on x[:, comp, :]: elem[p, i, k] = x[p, comp, i+k+1]."""
    pstep = tile_ap.ap[0][0]
    return _AP(tile_ap.tensor, tile_ap.offset + comp * 96 + 1,
               [[pstep, 128], [1, 64], [1, 32]])


def _bcast_pairs_ap(tile_ap, comp):
    """[128, 64, 32] view: elem[p, i, k] = x[p, comp, i]."""
    pstep = tile_ap.ap[0][0]
    return _AP(tile_ap.tensor, tile_ap.offset + comp * 96,
               [[pstep, 128], [1, 64], [0, 32]])


def _cosine_schedule(T, s=0.008):
    x = np.linspace(0.0, T, T + 1, dtype=np.float64)
    acp = np.cos(((x / T) + s) / (1.0 + s) * np.pi / 2.0) ** 2
    acp = acp / acp[0]
    betas = np.clip(1.0 - acp[1:] / acp[:-1], 1e-4, 0.999)
    alphas_cumprod = np.cumprod(1.0 - betas)
    return (np.sqrt(alphas_cumprod).astype(np.float32),
            np.sqrt(1.0 - alphas_cumprod).astype(np.float32))


SQRT_ACP, SQRT_OM_ACP = _cosine_schedule(TIMESTEPS)

_COMPILED = {}


def _build_program(reps=1, with_b2=False, use_silu=True, do_species=True, do_rep=True):
    nc = bacc.Bacc(None, target_bir_lowering=False)

    # ---- per-core external inputs ----
    ht = nc.dram_tensor("ht", [D, N_LOC], BF16, kind="ExternalInput")
    w1 = nc.dram_tensor("w1", [D, H], BF16, kind="ExternalInput")
    w2 = nc.dram_tensor("w2", [H, 128], BF16, kind="ExternalInput")
    w2p = nc.dram_tensor("w2p", [H, NT * W], BF16, kind="ExternalInput")
    b1c = nc.dram_tensor("b1c", [H, 1], F32, kind="ExternalInput")
    pmask = nc.dram_tensor("pmask", [128, NT * W], BF16, kind="ExternalInput")
    xs_d = nc.dram_tensor("xs_d", [B_LOC, 3 * 96], BF16, kind="ExternalInput")
    csc = nc.dram_tensor("csc", [B_LOC, 8], F32, kind="ExternalInput")
    eb2c = (nc.dram_tensor("eb2c", [128, C], BF16, kind="ExternalInput")
            if with_b2 else None)

    out = nc.dram_tensor("out", [128, 8], F32, kind="ExternalOutput")

    import contextlib
    with tile.TileContext(nc) as tc:
        rep_ctx = tc.For_i(0, reps, 1) if reps > 1 else contextlib.nullcontext()
        with (
            rep_ctx,
            tc.tile_pool(name="const", bufs=1) as cpool,
            tc.tile_pool(name="big", bufs=1) as bpool,
            tc.tile_pool(name="work", bufs=2) as wpool,
            tc.tile_pool(name="rep", bufs=2) as rpool,
        ):
            # ---------------- constants ----------------
            w1t = cpool.tile([D, H], BF16)
            nc.sync.dma_start(w1t[:], w1[:])
            w2t = cpool.tile([H, 128], BF16)
            nc.sync.dma_start(w2t[:], w2[:])
            w2pt = cpool.tile([H, NT, W], BF16)
            nc.sync.dma_start(w2pt[:], w2p[:].rearrange("h (t w) -> h t w", w=W))
            b1t = cpool.tile([H, 1], F32)
            nc.sync.dma_start(b1t[:], b1c[:])
            pmt = cpool.tile([128, NT, W], BF16)
            nc.sync.dma_start(pmt[:], pmask[:].rearrange("p (t w) -> p t w", w=W))
            if with_b2:
                eb2t = cpool.tile([128, C], BF16)
                nc.sync.dma_start(eb2t[:], eb2c[:])

            res = cpool.tile([128, 8], F32)
            nc.vector.memset(res[:, 6:8], 0.0)
            eps8 = cpool.tile([128, 1], F32)
            nc.vector.memset(eps8[:], 1e-8)
            pkcols = cpool.tile([128, NG], F32)
            sebuf = cpool.tile([128, NG, TPG], F32)

            # ---------------- repulsion (interleaved) ----------------
            rep_state = {}

            def emit_rep_dve(ct):
                sl = slice(ct * 128, (ct + 1) * 128)
                xs = rpool.tile([128, 3, 96], BF16, tag="xs")
                nc.sync.dma_start(xs[:], xs_d[sl, :].rearrange(
                    "p (c a) -> p c a", c=3))
                cs = rpool.tile([128, 8], F32, tag="cs")
                nc.sync.dma_start(cs[:], csc[sl, :])

                av = cs[:, 1:2]
                sq0 = cs[:, 2:3]; sq1 = cs[:, 3:4]
                u0 = cs[:, 4:5]; u1 = cs[:, 5:6]
                v0 = cs[:, 6:7]

                xst = xs[:]
                w0 = rpool.tile([128, 64, 32], BF16, tag="w0")
                nc.vector._custom_dve(
                    WRAP_SC, out=w0[:],
                    in0=_bcast_pairs_ap(xst, 0), in1=_shift_pairs_ap(xst, 0),
                    s0=av, s1=0.5)
                w1_ = rpool.tile([128, 64, 32], BF16, tag="w1_")
                nc.vector._custom_dve(
                    WRAP_DIFF, out=w1_[:],
                    in0=_bcast_pairs_ap(xst, 1), in1=_shift_pairs_ap(xst, 1),
                    s0=0.5)
                w2_ = rpool.tile([128, 64, 32], BF16, tag="w2_")
                nc.vector.tensor_tensor(
                    w2_[:], _bcast_pairs_ap(xst, 2), _shift_pairs_ap(xst, 2),
                    op=OP.subtract)

                # d2 = B + V;  B = (w0 + sq0*w1)^2 + sq1*w1^2
                #              V = (u0*w0 + u1*w1)*w2 + v0*w2^2
                w0f = w0[:].rearrange("p a b -> p (a b)")
                w1f = w1_[:].rearrange("p a b -> p (a b)")
                w2f = w2_[:].rearrange("p a b -> p (a b)")
                bq = rpool.tile([128, 2048], BF16, tag="bq")
                nc.vector._custom_dve(SQLC_PLUS, out=bq[:], in0=w0f,
                                      in1=w1f, s0=sq0, s1=sq1)
                uq = rpool.tile([128, 2048], BF16, tag="uq")
                nc.vector._custom_dve(LC2, out=uq[:], in0=w0f, in1=w1f,
                                      s0=u0, s1=u1)
                vq = rpool.tile([128, 2048], BF16, tag="vq")
                nc.vector._custom_dve(VSQW, out=vq[:], in0=uq[:], in1=w2f,
                                      s0=v0, s1=0.5)
                d2 = rpool.tile([128, 64, 32], BF16, tag="d2")
                d2f = d2[:].rearrange("p a b -> p (a b)")
                nc.vector.tensor_tensor(d2f, bq[:], vq[:], op=OP.add)
                # bf16 rounding can leave tiny negatives; ln needs >= 0
                nc.vector.tensor_scalar(d2f, d2f, 0.0, None, op0=OP.max)
                rep_state[ct] = d2

            def emit_rep_tail(ct):
                d2 = rep_state[ct]
                lnd = rpool.tile([128, 64, 32], BF16, tag="lnd")
                nc.scalar.activation(lnd[:], d2[:], AF.Ln,
                                     bias=eps8[:, 0:1], scale=1.0)
                dist = rpool.tile([128, 64, 32], BF16, tag="dist")
                nc.scalar.activation(dist[:], lnd[:], AF.Exp, scale=0.5)

                rscr = rpool.tile([128, 64, 32], BF16, tag="rscr")
                nc.vector._custom_dve(REP_TAIL, out=rscr[:], in0=dist[:],
                                      s0=0.0, s1=0.8,
                                      accum_out=res[:, 0 + ct:1 + ct])
                r32 = rpool.tile([128, 64], BF16, tag="r32")
                nc.vector._custom_dve(
                    REP_TAIL, out=r32[:], in0=dist[:, :, 31],
                    s0=0.0, s1=0.8,
                    accum_out=res[:, 2 + ct:3 + ct])

            # ---------------- species: mm1 + silu ----------------
            hidden = None
            if do_species:
                hidden = bpool.tile([H, N_LOC], BF16, tag="hidden")
            if do_species:
             with tc.tile_pool(name="psA", bufs=2, space="PSUM") as psA:
                for ch in range(NCH):
                    htc = wpool.tile([D, FCH], BF16, tag="htc")
                    nc.sync.dma_start(htc[:], ht[:, ch * FCH:(ch + 1) * FCH])
                    ps1 = psA.tile([H, FCH], F32, tag="ps1")
                    for j in range(FCH // 512):
                        nc.tensor.matmul(
                            ps1[:, j * 512:(j + 1) * 512],
                            w1t[:],
                            htc[:, j * 512:(j + 1) * 512],
                            start=True, stop=True,
                        )
                    if use_silu:
                        nc.scalar.activation(hidden[:, ch * FCH:(ch + 1) * FCH],
                                             ps1[:], AF.Silu,
                                             bias=b1t[:, 0:1], scale=1.0)
                    else:
                        sg = wpool.tile([H, FCH], BF16, tag="sg")
                        nc.scalar.activation(sg[:], ps1[:], AF.Sigmoid,
                                             bias=b1t[:, 0:1], scale=1.0)
                        nc.vector.scalar_tensor_tensor(
                            hidden[:, ch * FCH:(ch + 1) * FCH],
                            ps1[:], b1t[:, 0:1], sg[:],
                            op0=OP.add, op1=OP.mult)

            # ---------------- species: logits / lse / pick ----------------
            if not do_species:
                nc.vector.memset(sebuf[:], 1.0)
                nc.vector.memset(pkcols[:], 0.0)
            if do_rep:
                emit_rep_dve(0)
            with tc.tile_pool(name="psB", bufs=2, space="PSUM") as psB:
                for g in (range(NG) if do_species else []):
                    if do_rep and g == NG // 2:
                        emit_rep_tail(0)
                        emit_rep_dve(1)
                    lg = psB.tile([128, TPG, 128], F32, tag="lg")
                    for j in range(TPG):
                        at = g * TPG + j
                        hs = hidden[:, at * 128:(at + 1) * 128]
                        nc.tensor.matmul(lg[:, j, :], hs, w2t[:],
                                         start=True, stop=True)
                        nc.tensor.matmul(lg[:, j, C:C + W], hs, w2pt[:, at, :],
                                         start=True, stop=True)
                    # E = exp(logits) -> bf16, full contiguous tile; then
                    # zero the non-class columns (pick slots + garbage)
                    eg = wpool.tile([128, TPG, 128], BF16, tag="eg")
                    nc.scalar.activation(eg[:], lg[:], AF.Exp)
                    nc.gpsimd.memset(eg[:, :, C:128], 0.0)
                    if with_b2:
                        nc.vector.tensor_tensor(
                            eg[:, :, 0:C], eg[:, :, 0:C],
                            eb2t[:].unsqueeze(1).broadcast_to([128, TPG, C]),
                            op=OP.mult)
                    # per-atom sumexp: 128 -> 64 -> 32 -> reduce
                    f64 = wpool.tile([128, TPG, 64], BF16, tag="f64")
                    nc.vector.tensor_tensor(f64[:], eg[:, :, 0:64],
                                            eg[:, :, 64:128], op=OP.add)
                    f32t = wpool.tile([128, TPG, 32], BF16, tag="f32t")
                    nc.vector.tensor_tensor(f32t[:], f64[:, :, 0:32],
                                            f64[:, :, 32:64], op=OP.add)
                    nc.gpsimd.tensor_reduce(sebuf[:, g, :], f32t[:],
                                            axis=mybir.AxisListType.X, op=OP.add)
                    # pick: one-hot window mask (host) * pick logits, summed
                    pscr = wpool.tile([128, TPG, W], BF16, tag="pscr")
                    nc.vector.scalar_tensor_tensor(
                        pscr[:], pmt[:, g * TPG:(g + 1) * TPG, :], 0.0,
                        lg[:, :, C:C + W], op0=OP.bypass, op1=OP.mult,
                        accum_out=pkcols[:, g:g + 1])

            # sum over atoms of ln(sumexp)
            lnscr = cpool.tile([128, NG * TPG], F32)
            nc.scalar.activation(lnscr[:], sebuf[:].rearrange("p a b -> p (a b)"),
                                 AF.Ln, accum_out=res[:, 4:5])
            nc.vector.tensor_reduce(res[:, 5:6], pkcols[:],
                                    axis=mybir.AxisListType.X, op=OP.add)

            if do_rep:
                if not do_species:
                    emit_rep_dve(1)
                    emit_rep_tail(0)
                emit_rep_tail(1)

            nc.sync.dma_start(out[:], res[:])

    return nc


def _prep_inputs(inputs):
    f32 = np.float32
    frac = np.asarray(inputs["frac_coords"], f32)
    noise = np.asarray(inputs["noise"], f32)
    pn = np.asarray(inputs["pred_noise"], f32)
    h = np.asarray(inputs["h_final"], f32)
    lat = np.asarray(inputs["lattice"], f32)
    W1 = np.asarray(inputs["W1"], f32)
    b1 = np.asarray(inputs["b1"], f32)
    W2 = np.asarray(inputs["W2"], f32)
    b2 = np.asarray(inputs["b2"], f32)
    t = np.asarray(inputs["t"]).astype(np.int64)
    batch_indices = np.asarray(inputs["batch_indices"]).astype(np.int64)
    species = np.asarray(inputs["species"]).astype(np.int64)

    # ---- host: mse ----
    mse = float(np.mean((pn.astype(np.float64) - noise) ** 2))

    # ---- host: pred_x0 wrapped (match reference f32 ops) ----
    sa_b = SQRT_ACP[t]
    so_b = SQRT_OM_ACP[t]
    sa = sa_b[batch_indices][:, None].astype(f32)
    so = so_b[batch_indices][:, None].astype(f32)
    x_t = sa * frac + so * noise
    x_t = x_t - np.floor(x_t)
    px = (x_t - so * pn) / sa
    px = px - np.floor(px)                      # [N, 3]

    # ---- host: per-crystal quadratic-form scalars (Gram grouping) ----
    G = np.einsum("bkl,bml->bkm", lat.astype(np.float64),
                  lat.astype(np.float64))
    g00 = G[:, 0, 0]; g01 = G[:, 0, 1]; g02 = G[:, 0, 2]
    g11 = G[:, 1, 1]; g12 = G[:, 1, 2]; g22 = G[:, 2, 2]
    a = np.sqrt(g00)
    csc = np.stack([
        (a / 2.0), a,                       # wrap scale for comp0
        (g01 / a), (g11 - g01 * g01 / g00),  # sqlc_plus scalars
        (2.0 * g02 / a), (2.0 * g12),        # lc2 scalars
        g22, np.zeros_like(a),               # vsq scalar
    ], axis=1).astype(f32)                   # [B, 8]

    # coords deinterleaved + circular extension (comp0 scaled on device)
    pxc = px.reshape(B, NPER, 3).transpose(0, 2, 1).astype(f32)  # [B,3,64]
    pxc = np.concatenate([pxc, pxc[:, :, 0:32]], axis=2)         # [B,3,96]
    xs_d = np.ascontiguousarray(
        pxc.reshape(B, 3 * 96).astype(ml_dtypes.bfloat16))

    # ---- host: species head prep (global stable sort by species) ----
    order = np.argsort(species, kind="stable")
    ssort = species[order]
    hT = np.ascontiguousarray(h[order].T).astype(ml_dtypes.bfloat16)  # [64, N]
    w1b = W1.astype(ml_dtypes.bfloat16)
    w2b = np.concatenate([W2, np.zeros((H, 128 - C), f32)],
                         axis=1).astype(ml_dtypes.bfloat16)
    b1cv = b1.reshape(H, 1).astype(f32).copy()

    s_tiles = ssort.reshape(NCORES * NT, 128)
    cbase = s_tiles[:, 0]                          # class of first atom/tile
    assert (s_tiles.max(axis=1) - cbase).max() < W, "pick window too narrow"
    W2pad = np.concatenate([W2, np.zeros((H, W), f32)], axis=1)
    # w2p[core][h, t, w] = W2[h, cbase+w]
    w2p_all = np.stack([W2pad[:, cb:cb + W] for cb in cbase], axis=1)  # [H,NTtot,W]
    w2p_all = w2p_all.astype(ml_dtypes.bfloat16)
    # pick one-hot: pmask[core][p, t, w] = (ssort[tile t, atom p] == cbase+w)
    woff = (s_tiles - cbase[:, None])               # [NTtot, 128] in [0, W)
    pm_all = (woff[:, :, None] ==
              np.arange(W)[None, None, :]).astype(ml_dtypes.bfloat16)
    # -> [128, NTtot, W]
    pm_all = np.ascontiguousarray(pm_all.transpose(1, 0, 2))

    with_b2 = bool(np.any(b2))
    eb2c = (np.broadcast_to(np.exp(b2.astype(np.float64)).astype(np.float32),
                            (128, C)).astype(ml_dtypes.bfloat16)
            if with_b2 else None)
    host_b2s = float(b2[species].sum(dtype=np.float64))

    in_maps = []
    for c in range(NCORES):
        asl = slice(c * N_LOC, (c + 1) * N_LOC)
        tsl = slice(c * NT, (c + 1) * NT)
        bsl = slice(c * B_LOC, (c + 1) * B_LOC)
        in_maps.append({
            "ht": np.ascontiguousarray(hT[:, asl]),
            "w1": w1b, "w2": w2b, "b1c": b1cv,
            "w2p": np.ascontiguousarray(
                w2p_all[:, tsl, :].reshape(H, NT * W)),
            "pmask": np.ascontiguousarray(
                pm_all[:, tsl, :].reshape(128, NT * W)),
            "xs_d": np.ascontiguousarray(xs_d[bsl]),
            "csc": np.ascontiguousarray(csc[bsl]),
            **({"eb2c": eb2c} if with_b2 else {}),
        })
    return in_maps, (mse, host_b2s), with_b2


def kernel(**inputs) -> tuple:
    in_maps, (mse, host_b2s), with_b2 = _prep_inputs(inputs)
    key = ("prog", with_b2)
    if key not in _COMPILED:
        _COMPILED[key] = _build_program(with_b2=with_b2)
        _COMPILED[key].compile()
    nc = _COMPILED[key]
    res = run_bass_kernel_spmd(nc, in_maps, list(range(NCORES)))
    outs = [r["out"] for r in res.results]

    rep_total = 0.0
    lse_total = 0.0
    pick_total = 0.0
    for o in outs:
        o = o.astype(np.float64)
        for ct in range(CT):
            rep_total += (2.0 * o[:, 0 + ct] - o[:, 2 + ct]).sum()
        lse_total += o[:, 4].sum()
        pick_total += o[:, 5].sum()

    l_rep = rep_total / NPER / B
    loss_diffusion = np.float32(mse + 5.0 * l_rep)
    loss_species = np.float32((lse_total - (pick_total + host_b2s)) / N)
    l_repulsion = np.float32(l_rep)
    return (loss_diffusion, loss_species, l_repulsion)


if __name__ == "__main__":
    import reference as ref
    inputs = {k: np.asarray(v) for k, v in ref.setup_inputs().items()}
    got = kernel(**inputs)
    print("kernel:", got)
